# revision 14
# baseline (speedup 1.0000x reference)
"""ContextNorm (training-mode per-context batch-norm) on 8 Trainium2 cores.

Strategy (data parallel over rows, per sharding hint):
  - 125,000 rows per core, padded to 245 superblocks x 512 rows (pad ids = 16,
    which matches no context -> contributes nothing to stats).
  - Pass 1: per-core segment sums via one-hot matmul on the TensorEngine.
    Moving operand is [x | x^2] in bf16 (error averages out over ~62k rows per
    context); stationary is the one-hot [128 rows, 4*16] built with is_equal.
    Counts accumulate via a ones-column matmul. All accumulation in fp32 PSUM.
  - AllReduce of the [16,129] (sum|sumsq|count) stats across the 8 cores.
  - Compute scale = gamma' * rsqrt(var+eps), shift = beta' - scale*mean where
    gamma' = gamma/sqrt(prior), beta' = beta/sqrt(prior) (host-precomputed,
    O(C*D)).
  - Pass 2: per-row gather of [scale|shift] via (one-hot^T @ block-diag ss)
    matmul in fp32 (exact), then y = x*rowscale + rowshift on the vector
    engine, streamed back out.

Self-contained: hardcodes shapes from the problem spec.
"""

import numpy as np

from concourse import bacc, bass, mybir, tile
from concourse import bass2jax

# Problem constants (from spec).
N, D, C = 1_000_000, 64, 16
EPS = 1e-3
NCORES = 8
NPC = N // NCORES  # 125_000 rows per core

# Tiling.
G = 4  # row-groups per superblock (rows interleaved 4-per-partition)
SB = 128 * G  # 512 rows per superblock
FP32 = mybir.dt.float32
BF16 = mybir.dt.bfloat16
INT8 = mybir.dt.int8


def _build(npc: int, bb: int, n_cores: int = NCORES):
    """Build + compile the bass program for `npc` rows/core, `bb` superblocks
    per DMA batch. npc must be divisible by SB*bb."""
    nsb = npc // SB
    assert nsb * SB == npc and nsb % bb == 0
    nb = nsb // bb

    nc = bacc.Bacc(
        "TRN2",
        target_bir_lowering=False,
        debug=False,
        enable_asserts=False,
        num_devices=n_cores,
    )

    x_d = nc.dram_tensor("x", [npc, D], FP32, kind="ExternalInput").ap()
    # ids replicated for the pass-1 one-hot: [128, nsb*64] where
    # [p, b*64 + 16g + c] = id(row b*512 + 4p + g), replicated over c.
    idsp_d = nc.dram_tensor("idsp", [128, nsb * 64], INT8, kind="ExternalInput").ap()
    # ids replicated for the pass-2 one-hot^T: [64, nsb*128] where
    # [16g + c, b*128 + p] = id(row b*512 + 4p + g), replicated over c.
    idst_d = nc.dram_tensor("idst", [64, nsb * 128], INT8, kind="ExternalInput").ap()
    # iota context pattern for pass-1: [128, bb*64], [p, s*64+16g+c] = c.
    iota1_d = nc.dram_tensor("iota1", [128, bb * 64], INT8, kind="ExternalInput").ap()
    # iota context pattern for pass-2: [64, 1], [16g+c, 0] = c.
    iota2_d = nc.dram_tensor("iota2", [64, 1], FP32, kind="ExternalInput").ap()
    # [16, 128] = [gamma/sqrt(prior) | beta/sqrt(prior)].
    gb_d = nc.dram_tensor("gb", [16, 128], FP32, kind="ExternalInput").ap()
    y_d = nc.dram_tensor("y", [npc, D], FP32, kind="ExternalOutput").ap()

    # Row-major [npc, 64] viewed as [128 part, superblock, 4*64], where
    # partition p of superblock b holds rows b*512 + 4p + {0..3} (1KB contig).
    x_v = x_d.rearrange("(b p g) d -> p b (g d)", p=128, g=G)
    y_v = y_d.rearrange("(b p g) d -> p b (g d)", p=128, g=G)

    with tile.TileContext(nc) as tc:
        with (
            tc.tile_pool(name="const", bufs=1) as cpool,
            tc.tile_pool(name="work", bufs=6) as xpool,
            tc.tile_pool(name="mov", bufs=3) as movpool,
            tc.tile_pool(name="oh", bufs=3) as ohpool,
            tc.tile_pool(name="yow", bufs=3) as ypool,
            tc.tile_pool(name="small", bufs=1) as spool,
            tc.tile_pool(name="psum", bufs=1, space="PSUM") as pspool,
            tc.tile_pool(name="psumrv", bufs=4, space="PSUM") as rvpool,
            tc.tile_pool(name="dram", bufs=1, space="DRAM") as dpool,
        ):
            # ---- constants ----
            idsp_sb = cpool.tile([128, nsb * 64], INT8, tag="idsp")
            nc.sync.dma_start(out=idsp_sb[:], in_=idsp_d[:])
            idst_sb = cpool.tile([64, nsb * 128], INT8, tag="idst")
            nc.sync.dma_start(out=idst_sb[:], in_=idst_d[:])
            iota1_sb = cpool.tile([128, bb * 64], INT8, tag="iota1")
            nc.sync.dma_start(out=iota1_sb[:], in_=iota1_d[:])
            iota2_sb = cpool.tile([64, 1], FP32, tag="iota2")
            nc.sync.dma_start(out=iota2_sb[:], in_=iota2_d[:])
            gb_sb = cpool.tile([16, 128], FP32, tag="gb")
            nc.sync.dma_start(out=gb_sb[:], in_=gb_d[:])
            ones_sb = cpool.tile([128, 1], BF16, tag="ones")
            nc.vector.memset(ones_sb[:], 1.0)

            # ---- pass 1: segment sums ----
            # stats_ps[c, 128g + t*64 + d] = sum over rows of group g with
            # context c of (x if t==0 else x^2); cnt_ps[c, g] = row count.
            stats_ps = pspool.tile([16, 512], FP32, tag="stats")
            cnt_ps = pspool.tile([16, G], FP32, tag="cnt")
            for jb in range(nb):
                xt = xpool.tile([128, bb * 256], FP32, tag="xt")
                nc.sync.dma_start(
                    out=xt[:].rearrange("p (b f) -> p b f", b=bb),
                    in_=x_v[:, jb * bb : (jb + 1) * bb, :],
                )
                mov = movpool.tile([128, bb * 512], BF16, tag="mov")
                # halves: [0 : bb*256] = x (bf16), [bb*256 :] = x^2 (bf16)
                nc.vector.tensor_copy(out=mov[:, 0 : bb * 256], in_=xt[:])
                nc.scalar.square(out=mov[:, bb * 256 : bb * 512], in_=xt[:])
                oh1 = ohpool.tile([128, bb * 64], BF16, tag="oh1")
                nc.vector.tensor_tensor(
                    out=oh1[:],
                    in0=idsp_sb[:, jb * bb * 64 : (jb + 1) * bb * 64],
                    in1=iota1_sb[:],
                    op=mybir.AluOpType.is_equal,
                )
                mov_v = mov[:].rearrange("p (t b g d) -> p b g t d", t=2, b=bb, g=G)
                for s in range(bb):
                    for g in range(G):
                        # one accumulation group per PSUM bank: only the very
                        # first matmul zeros the bank, the very last stops.
                        first = jb == 0 and s == 0 and g == 0
                        last = jb == nb - 1 and s == bb - 1 and g == G - 1
                        lhs = oh1[:, s * 64 + 16 * g : s * 64 + 16 * g + 16]
                        nc.tensor.matmul(
                            out=stats_ps[:, 128 * g : 128 * g + 128],
                            lhsT=lhs,
                            rhs=mov_v[:, s, g],
                            start=first,
                            stop=last,
                        )
                        nc.tensor.matmul(
                            out=cnt_ps[:, g : g + 1],
                            lhsT=lhs,
                            rhs=ones_sb[:],
                            start=first,
                            stop=last,
                        )

            # ---- finalize + all-reduce stats ----
            add = mybir.AluOpType.add
            stats_sb = spool.tile([16, 512], FP32, tag="statss")
            nc.scalar.activation(
                out=stats_sb[:], in_=stats_ps[:], func=mybir.ActivationFunctionType.Copy
            )
            cnt_sb = spool.tile([16, G], FP32, tag="cnts")
            nc.scalar.activation(
                out=cnt_sb[:], in_=cnt_ps[:], func=mybir.ActivationFunctionType.Copy
            )
            # fold groups: red[:, t*64+d] = sum_g stats_sb[:, 128g + t*64 + d]
            red = spool.tile([16, 129], FP32, tag="red")
            t01 = spool.tile([16, 128], FP32, tag="t01")
            t23 = spool.tile([16, 128], FP32, tag="t23")
            sv = stats_sb[:].rearrange("c (g f) -> c g f", g=G)
            nc.vector.tensor_tensor(out=t01[:], in0=sv[:, 0], in1=sv[:, 1], op=add)
            nc.vector.tensor_tensor(out=t23[:], in0=sv[:, 2], in1=sv[:, 3], op=add)
            nc.vector.tensor_tensor(out=red[:, 0:128], in0=t01[:], in1=t23[:], op=add)
            nc.vector.tensor_tensor(
                out=t01[:, 0:1], in0=cnt_sb[:, 0:1], in1=cnt_sb[:, 1:2], op=add
            )
            nc.vector.tensor_tensor(
                out=t23[:, 0:1], in0=cnt_sb[:, 2:3], in1=cnt_sb[:, 3:4], op=add
            )
            nc.vector.tensor_tensor(
                out=red[:, 128:129], in0=t01[:, 0:1], in1=t23[:, 0:1], op=add
            )

            bnc_in = dpool.tile([16, 129], FP32, tag="bin")
            bnc_out = dpool.tile([16, 129], FP32, tag="bout", addr_space="Shared")
            nc.sync.dma_start(out=bnc_in[:], in_=red[:])
            if n_cores > 1:
                nc.gpsimd.collective_compute(
                    "AllReduce",
                    mybir.AluOpType.add,
                    replica_groups=[list(range(n_cores))],
                    ins=[bnc_in[:]],
                    outs=[bnc_out[:]],
                )
                gsrc = bnc_out
            else:
                gsrc = bnc_in
            gstats = spool.tile([16, 129], FP32, tag="gstats")
            nc.sync.dma_start(out=gstats[:], in_=gsrc[:])

            # ---- scale/shift ----
            mul = mybir.AluOpType.mult
            sub = mybir.AluOpType.subtract
            sw = spool.tile([16, 64], FP32, tag="sw")  # scratch
            rec = spool.tile([16, 1], FP32, tag="rec")
            mean = spool.tile([16, 64], FP32, tag="mean")
            ex2 = spool.tile([16, 64], FP32, tag="ex2")
            istd = spool.tile([16, 64], FP32, tag="istd")
            ss = spool.tile([16, 128], FP32, tag="ss")
            nc.vector.tensor_scalar_max(sw[:, 0:1], gstats[:, 128:129], 1.0)
            nc.vector.reciprocal(rec[:], sw[:, 0:1])
            nc.vector.tensor_scalar_mul(mean[:], gstats[:, 0:64], rec[:])
            nc.vector.tensor_scalar_mul(ex2[:], gstats[:, 64:128], rec[:])
            nc.vector.tensor_tensor(out=sw[:], in0=mean[:], in1=mean[:], op=mul)
            nc.vector.tensor_tensor(out=sw[:], in0=ex2[:], in1=sw[:], op=sub)  # var
            nc.vector.tensor_scalar_add(sw[:], sw[:], EPS)
            nc.scalar.sqrt(out=sw[:], in_=sw[:])
            nc.vector.reciprocal(istd[:], sw[:])
            # scale = gamma' * istd ; shift = beta' - scale * mean
            nc.vector.tensor_tensor(
                out=ss[:, 0:64], in0=gb_sb[:, 0:64], in1=istd[:], op=mul
            )
            nc.vector.tensor_tensor(out=sw[:], in0=ss[:, 0:64], in1=mean[:], op=mul)
            nc.vector.tensor_tensor(
                out=ss[:, 64:128], in0=gb_sb[:, 64:128], in1=sw[:], op=sub
            )
            # block-diagonal [64, 512]: rows 16g:16g+16, cols 128g:128g+128 = ss
            ss4 = spool.tile([64, 512], FP32, tag="ss4")
            nc.vector.memset(ss4[:], 0.0)
            for g in range(G):
                # compute engines can't start at partition 16; DMA can
                nc.sync.dma_start(
                    out=ss4[16 * g : 16 * g + 16, 128 * g : 128 * g + 128],
                    in_=ss[:],
                )

            # ---- pass 2: gather + affine ----
            for jb in range(nb):
                xt = xpool.tile([128, bb * 256], FP32, tag="xt")
                nc.sync.dma_start(
                    out=xt[:].rearrange("p (b f) -> p b f", b=bb),
                    in_=x_v[:, jb * bb : (jb + 1) * bb, :],
                )
                oht = ohpool.tile([64, bb * 128], FP32, tag="oht")
                nc.gpsimd.tensor_scalar(
                    out=oht[:],
                    in0=idst_sb[:, jb * bb * 128 : (jb + 1) * bb * 128],
                    scalar1=iota2_sb[:],
                    scalar2=None,
                    op0=mybir.AluOpType.is_equal,
                )
                yt = ypool.tile([128, bb * 256], FP32, tag="yt")
                for s in range(bb):
                    rv = rvpool.tile([128, 512], FP32, tag="rv")
                    nc.tensor.matmul(
                        out=rv[:],
                        lhsT=oht[:, s * 128 : (s + 1) * 128],
                        rhs=ss4[:],
                        start=True,
                        stop=True,
                    )
                    # y = x * rowscale + rowshift
                    rv_v = rv[:].rearrange("p (g t d) -> p g t d", g=G, t=2)
                    x_s = xt[:, s * 256 : (s + 1) * 256].rearrange(
                        "p (g d) -> p g d", g=G
                    )
                    y_s = yt[:, s * 256 : (s + 1) * 256].rearrange(
                        "p (g d) -> p g d", g=G
                    )
                    nc.vector.tensor_tensor(
                        out=y_s, in0=x_s, in1=rv_v[:, :, 0], op=mul
                    )
                    nc.vector.tensor_tensor(
                        out=y_s, in0=y_s, in1=rv_v[:, :, 1], op=add
                    )
                nc.sync.dma_start(
                    out=y_v[:, jb * bb : (jb + 1) * bb, :],
                    in_=yt[:].rearrange("p (b f) -> p b f", b=bb),
                )

    nc.compile()
    return nc


def _prep_ids(ids_pad: np.ndarray, nsb: int, bb: int):
    """ids_pad: [nsb*512] int -> (idsp [128, nsb*64] int8, idst [64, nsb*128]
    int8) replicated one-hot comparison operands."""
    a = ids_pad.reshape(nsb, 128, G).astype(np.int8)  # [b, p, g]
    # idsp[p, b*64 + 16g + c] = a[b, p, g]
    idsp = np.repeat(a.transpose(1, 0, 2), 16, axis=2).reshape(128, nsb * 64)
    # idst[16g + c, b*128 + p] = a[b, p, g]
    idst = (
        np.repeat(a.transpose(2, 0, 1)[:, None], 16, axis=1)
        .transpose(0, 1, 2, 3)
        .reshape(64, nsb * 128)
    )
    return np.ascontiguousarray(idsp), np.ascontiguousarray(idst)


def _iotas(bb: int):
    iota1 = np.tile(
        np.tile(np.arange(16, dtype=np.int8), G)[None, :], (128, bb)
    ).reshape(128, bb * 64)
    iota2 = np.tile(np.arange(16, dtype=np.float32), G).reshape(64, 1)
    return iota1, iota2


def make_in_maps(samples, contexts, gamma, beta, priors, npc, bb, n_cores):
    nsb = npc // SB
    pscale = 1.0 / np.sqrt(priors.astype(np.float64))
    gp = (gamma.astype(np.float64) * pscale[:, None]).astype(np.float32)
    bp = (beta.astype(np.float64) * pscale[:, None]).astype(np.float32)
    gb = np.ascontiguousarray(np.concatenate([gp, bp], axis=1))  # [16,128]
    iota1, iota2 = _iotas(bb)
    samples = np.asarray(samples)
    contexts = np.asarray(contexts)
    n_real = samples.shape[0] // n_cores
    in_maps = []
    for i in range(n_cores):
        xs = samples[i * n_real : (i + 1) * n_real]
        ids = contexts[i * n_real : (i + 1) * n_real, 0]
        x_pad = np.zeros((npc, D), np.float32)
        x_pad[:n_real] = xs
        ids_pad = np.full((npc,), 16, np.int32)
        ids_pad[:n_real] = ids
        idsp, idst = _prep_ids(ids_pad, nsb, bb)
        in_maps.append(
            {
                "x": x_pad,
                "idsp": idsp,
                "idst": idst,
                "iota1": iota1,
                "iota2": iota2,
                "gb": gb,
            }
        )
    return in_maps


# Full-size program cache (compile once per process).
_CACHE: dict = {}

NPC_PAD = 245 * SB  # 125_440
BB = 7  # 245 superblocks = 35 batches of 7


def get_program(npc=NPC_PAD, bb=BB, n_cores=NCORES):
    key = (npc, bb, n_cores)
    if key not in _CACHE:
        _CACHE[key] = _build(npc, bb, n_cores)
    return _CACHE[key]


def kernel(samples, contexts, gamma, beta, priors):
    samples = np.asarray(samples, np.float32)
    contexts = np.asarray(contexts, np.int32)
    gamma = np.asarray(gamma, np.float32)
    beta = np.asarray(beta, np.float32)
    priors = np.asarray(priors, np.float32)
    nc = get_program()
    in_maps = make_in_maps(
        samples, contexts, gamma, beta, priors, NPC_PAD, BB, NCORES
    )
    outs = bass2jax.run_bass_via_pjrt(nc, in_maps, n_cores=NCORES)
    n_real = N // NCORES
    y = np.concatenate([outs[i]["y"][:n_real] for i in range(NCORES)], axis=0)
    return y.astype(np.float32)
